# revision 1
# baseline (speedup 1.0000x reference)
"""Causal self-attention kernel for Trainium2, 8-core data parallel.

Per-core program: one batch element b of x [8, 1024, 768].
  - x, W_attn, W_proj transposed into c-on-partitions layouts via
    cast-to-bf16 + DRAM round-trip DMA transpose.
  - qkT (Q features) / KTpad (K features, zero-padded to 128 rows for FWL)
    / V [t, j]: weight-reuse-ordered GEMMs.
  - per head: S^T = K^T.T @ Q^T (k on partitions, q free), P^T = exp(S^T/8)
    with causal tri-mask on diagonal blocks; y^T_h = V^T-slices @ P^T with a
    concurrent col-tiled ones-matmul accumulating softmax denominators;
    normalized via PE-broadcast reciprocal.
  - out = y @ Wp^T + b_p (t on partitions).
All matmuls bf16 inputs / fp32 PSUM accumulation; softmax in fp32.
"""
import sys
import contextlib
from contextlib import ExitStack

sys.path.insert(0, "/opt/trn_rl_repo")

import numpy as np

import concourse.bass as bass
import concourse.bacc as bacc
import concourse.mybir as mybir
import concourse.tile as tile
from concourse.masks import make_upper_triangular

F32 = mybir.dt.float32
BF16 = mybir.dt.bfloat16

P = 128
T = 1024
C = 768
H = 12
HS = 64
CT = C // P     # 6 c-tiles
TT = T // P     # 8 t-tiles
KT = T // P     # 8 k-tiles per head
N_CORES = 8


def qk_chunks(kt):
    """[(q0, w)] matmul chunks for k-tile kt: q >= kt*128, each split in two
    for stationary-weight reuse."""
    qs = kt * P
    if kt < 4:
        return [(qs, 512 - qs), (512, 512)]
    w = T - qs
    h0 = (w // 2) // 64 * 64
    return [(qs, h0), (qs + h0, w - h0)]


def pv_chunks(kt, b):
    """[(off, w)] chunks within q-block b for k-tile kt."""
    off = max(0, kt * P - b * 512)
    return [(off, 512 - off)]


def emit_consts(nc, tc, const, ba_d, bp_d):
    from concourse.masks import make_identity
    ident = const.tile([P, P], F32, tag="ident")
    make_identity(nc, ident)
    tri = const.tile([P, P], BF16, tag="tri")
    make_upper_triangular(nc, tri, val=1.0, diag=True)
    ones_bf = const.tile([P, P], BF16, tag="ones")
    nc.gpsimd.memset(ones_bf[:], 1.0)
    bias_qk = const.tile([P, 12], F32, tag="bqk")
    nc.sync.dma_start(bias_qk[:], ba_d[0 : 2 * C].rearrange("(o p) -> p o", p=P))
    Bv = const.tile([P, C], F32, tag="Bv")
    nc.sync.dma_start(
        Bv[:],
        ba_d[2 * C : 3 * C].rearrange("(a j) -> a j", a=1).to_broadcast([P, C]),
    )
    Bp = const.tile([P, C], F32, tag="Bp")
    nc.sync.dma_start(
        Bp[:], bp_d.rearrange("(a j) -> a j", a=1).to_broadcast([P, C])
    )
    return dict(ident=ident, tri=tri, ones_bf=ones_bf, bias_qk=bias_qk, Bv=Bv, Bp=Bp)


def emit_body(nc, tc, pools, cst, dram, x_d, wa_d, wp_d, y_d):
    const, persist, nat, work, ptp, ps512, ps384 = pools
    tri, ones_bf = cst["tri"], cst["ones_bf"]
    bias_qk, Bv, Bp = cst["bias_qk"], cst["Bv"], cst["Bp"]

    xT = persist.tile([P, CT, T], BF16, tag="xT")
    WT = persist.tile([P, CT, 3 * C], BF16, tag="WT")
    WpT = persist.tile([P, CT, C], BF16, tag="WpT")
    qT = persist.tile([P, CT, T], BF16, tag="qT")         # Q features, pair layout
    KTp = persist.tile([P, 12, T], BF16, tag="KTp")       # per-head, zero-padded
    V = persist.tile([P, TT, 12, 65], BF16, tag="Vaug")
    yT = persist.tile([P, CT, T], BF16, tag="yT")

    # ---- phase 0: transpose x, W_attn, W_proj via PE (identity matmul)
    ident = cst["ident"]

    def transpose_in(dram_src, n_row_tiles, dst):
        src = dram_src.rearrange("(rt p) c -> rt p c", p=P)
        for rt in range(n_row_tiles):
            natt = nat.tile([P, C], F32, tag="nat")
            nc.sync.dma_start(natt[:], src[rt])
            for g in range(2):
                ps = ps512.tile([P, 512], F32, tag="ps512", name="tps")[:, :384]
                for i in range(3):
                    ct = g * 3 + i
                    nc.tensor.transpose(
                        ps[:, i * P : (i + 1) * P],
                        natt[:, ct * P : (ct + 1) * P],
                        ident[:],
                    )
                eng = nc.vector if rt % 2 == 0 else nc.scalar
                dstap = dst[:, g * 3 : (g + 1) * 3, rt * P : (rt + 1) * P]
                srcap = ps[:].rearrange("p (a b) -> p a b", a=3)
                if eng is nc.vector:
                    nc.vector.tensor_copy(dstap, srcap)
                else:
                    nc.scalar.activation(
                        dstap, srcap, mybir.ActivationFunctionType.Copy
                    )

    transpose_in(x_d, TT, xT)
    transpose_in(wa_d, 18, WT)
    transpose_in(wp_d, CT, WpT)

    # ---- phase 1: Q -> qT (pair layout), K -> KTp (padded per head), V [t, j]
    for jt in (0, 6, 1, 7, 2, 8, 3, 9, 4, 10, 5, 11):
        pss = [ps512.tile([P, 512], F32, tag="ps512", name=f"qkps{i}") for i in range(2)]
        for ct in range(CT):
            for tb in range(2):
                nc.tensor.matmul(
                    pss[tb][:],
                    WT[:, ct, jt * P : (jt + 1) * P],
                    xT[:, ct, tb * 512 : (tb + 1) * 512],
                    start=(ct == 0),
                    stop=(ct == CT - 1),
                )
        for tb in range(2):
            sl = slice(tb * 512, (tb + 1) * 512)
            if jt < 6:
                nc.scalar.activation(
                    qT[:, jt, sl], pss[tb][:],
                    mybir.ActivationFunctionType.Identity,
                    bias=bias_qk[:, jt : jt + 1],
                )
            else:
                j = jt - 6
                nc.vector.tensor_scalar_add(
                    KTp[0:64, 2 * j, sl], pss[tb][0:64, :],
                    bias_qk[0:64, jt : jt + 1],
                )
                nc.vector.tensor_scalar_add(
                    KTp[64:128, 2 * j + 1, sl], pss[tb][64:128, :],
                    bias_qk[64:128, jt : jt + 1],
                )
    for tt in range(TT):
        pss = [ps512.tile([P, 512], F32, tag="ps512", name=f"vps{i}")[:, :384] for i in range(2)]
        for ct in range(CT):
            for jb in range(2):
                nc.tensor.matmul(
                    pss[jb][:],
                    xT[:, ct, tt * P : (tt + 1) * P],
                    WT[:, ct, 2 * C + jb * 384 : 2 * C + (jb + 1) * 384],
                    start=(ct == 0),
                    stop=(ct == CT - 1),
                )
        for jb in range(2):
            sl = slice(jb * 384, (jb + 1) * 384)
            nc.vector.tensor_add(
                V[:, tt, 6 * jb : 6 * jb + 6, 0:64],
                pss[jb][:].rearrange("p (h d) -> p h d", d=64),
                Bv[:, sl].rearrange("p (h d) -> p h d", d=64),
            )

    # ---- phase 2: per-head attention (QK emitted one head ahead of PV)
    PTs = {}

    def emit_qk_head(h):
        hb = 64 * (h % 2)
        qj = h // 2
        PT = ptp.tile([P, KT, T], BF16, tag="PT", name=f"PT{h}")
        PTs[h] = PT
        for kt in range(KT):
            qs = kt * P
            for (q0, w) in qk_chunks(kt):
                sps = ps512.tile([P, 512], F32, tag="ps512", name="sps")
                nc.tensor.matmul(
                    sps[:, :w],
                    KTp[:, h, kt * P : (kt + 1) * P],
                    qT[:, qj, q0 : q0 + w],
                    start=True,
                    stop=True,
                )
                nc.scalar.activation(
                    PT[:, kt, q0 : q0 + w],
                    sps[:, :w],
                    mybir.ActivationFunctionType.Exp,
                    scale=0.125,
                )
            nc.gpsimd.affine_select(
                out=PT[:, kt, qs : qs + P],
                in_=PT[:, kt, qs : qs + P],
                compare_op=mybir.AluOpType.is_ge,
                fill=0.0,
                base=0,
                # keep where (-k + q') >= 0, else fill 0
                pattern=[[1, P]],
                channel_multiplier=-1,
            )

    def emit_pv_head(h):
        hb = 64 * (h % 2)
        PT = PTs.pop(h)
        yDs = [
            ps512.tile([P, 512], F32, tag="ps512", name=f"yD{i}")
            for i in range(2)
        ]
        for kt in range(KT):
            for b in range(2):
                if b == 0 and kt >= 4:
                    continue
                (off, w), = pv_chunks(kt, b)
                first = kt == 0
                last = kt == (3 if b == 0 else KT - 1)
                rhs = PT[:, kt, b * 512 + off : b * 512 + off + w]
                nc.tensor.matmul(
                    yDs[b][0:65, off : off + w],
                    V[:, kt, h, :],
                    rhs,
                    start=first,
                    stop=last,
                )
        for b in range(2):
            yD = yDs[b]
            Dr = work.tile([P, 512], F32, tag="Dr")
            nc.vector.reciprocal(Dr[64:65, :], yD[64:65, :])
            Drb = work.tile([P, 512], BF16, tag="Drb")
            nc.vector.tensor_copy(Drb[64:65, :], Dr[64:65, :])
            rps = ps512.tile([P, 512], F32, tag="ps512", name="rps")
            nc.tensor.matmul(
                rps[0:64, :],
                ones_bf[64:65, 0:64],
                Drb[64:65, :],
                start=True,
                stop=True,
                tile_position=(64, 0),
            )
            Rh = work.tile([P, 512], F32, tag="Rh")
            nc.scalar.activation(
                Rh[0:64, :],
                rps[0:64, :],
                mybir.ActivationFunctionType.Copy,
            )
            if hb == 0:
                nc.vector.tensor_mul(
                    yT[0:64, h // 2, b * 512 : (b + 1) * 512],
                    yD[0:64, :],
                    Rh[0:64, :],
                )
            else:
                yTt = work.tile([P, 512], BF16, tag="yTt")
                nc.vector.tensor_mul(yTt[0:64, :], yD[0:64, :], Rh[0:64, :])
                nc.sync.dma_start(
                    yT[64:128, h // 2, b * 512 : (b + 1) * 512], yTt[0:64, :]
                )

    emit_qk_head(0)
    for h in range(H):
        if h + 1 < H:
            emit_qk_head(h + 1)
        emit_pv_head(h)

    # ---- phase 3: out = y @ Wp^T + b_p
    for tt in range(TT):
        osb = work.tile([P, C], F32, tag="osb")
        pss = [ps512.tile([P, 512], F32, tag="ps512", name=f"vps{i}")[:, :384] for i in range(2)]
        for ct in range(CT):
            for jb in range(2):
                nc.tensor.matmul(
                    pss[jb][:],
                    yT[:, ct, tt * P : (tt + 1) * P],
                    WpT[:, ct, jb * 384 : (jb + 1) * 384],
                    start=(ct == 0),
                    stop=(ct == CT - 1),
                )
        for jb in range(2):
            sl = slice(jb * 384, (jb + 1) * 384)
            nc.vector.tensor_add(osb[:, sl], pss[jb][:], Bp[:, sl])
        nc.sync.dma_start(
            y_d.rearrange("(tt p) c -> tt p c", p=P)[tt], osb[:]
        )


def build_program(loop=1):
    nc = bacc.Bacc("TRN2", target_bir_lowering=False, debug=False)
    x_d = nc.dram_tensor("x", [T, C], F32, kind="ExternalInput").ap()
    wa_d = nc.dram_tensor("W_attn", [3 * C, C], F32, kind="ExternalInput").ap()
    ba_d = nc.dram_tensor("b_attn", [3 * C], F32, kind="ExternalInput").ap()
    wp_d = nc.dram_tensor("W_proj", [C, C], F32, kind="ExternalInput").ap()
    bp_d = nc.dram_tensor("b_proj", [C], F32, kind="ExternalInput").ap()
    y_d = nc.dram_tensor("y", [T, C], F32, kind="ExternalOutput").ap()

    with tile.TileContext(nc) as tc, ExitStack() as ctx:
        const = ctx.enter_context(tc.tile_pool(name="const", bufs=1))
        persist = ctx.enter_context(tc.tile_pool(name="persist", bufs=1))
        nat = ctx.enter_context(tc.tile_pool(name="nat", bufs=6))
        work = ctx.enter_context(tc.tile_pool(name="work", bufs=2))
        ptp = ctx.enter_context(tc.tile_pool(name="ptp", bufs=3))
        dram = ctx.enter_context(tc.tile_pool(name="dram", bufs=1, space="DRAM"))
        ps512 = ctx.enter_context(tc.tile_pool(name="ps512", bufs=8, space="PSUM"))
        ps384 = ps512
        pools = (const, persist, nat, work, ptp, ps512, ps384)

        cst = emit_consts(nc, tc, const, ba_d, bp_d)
        # zero-fill KTp's complementary halves + V ones columns once
        KTp0 = persist.tile([P, 12, T], BF16, tag="KTp")
        nc.gpsimd.memset(KTp0[:], 0.0)
        V0 = persist.tile([P, TT, 12, 65], BF16, tag="Vaug")
        nc.gpsimd.memset(V0[:, :, :, 64:65], 1.0)
        loop_cm = tc.For_i(0, loop, 1) if loop > 1 else contextlib.nullcontext()
        with loop_cm:
            emit_body(nc, tc, pools, cst, dram, x_d, wa_d, wp_d, y_d)

    nc.compile()
    return nc


_CACHED_NC = None


def kernel(x, W_attn, b_attn, W_proj, b_proj):
    from concourse.bass_utils import run_bass_kernel_spmd

    global _CACHED_NC
    if _CACHED_NC is None:
        _CACHED_NC = build_program(loop=1)
    nc = _CACHED_NC

    B = x.shape[0]
    assert B == N_CORES
    in_maps = [
        {
            "x": np.ascontiguousarray(x[b], dtype=np.float32),
            "W_attn": np.asarray(W_attn, dtype=np.float32),
            "b_attn": np.asarray(b_attn, dtype=np.float32),
            "W_proj": np.asarray(W_proj, dtype=np.float32),
            "b_proj": np.asarray(b_proj, dtype=np.float32),
        }
        for b in range(B)
    ]
    res = run_bass_kernel_spmd(nc, in_maps, list(range(N_CORES)))
    return np.stack([res.results[b]["y"] for b in range(B)], axis=0)



# revision 17
# speedup vs baseline: 1.3351x; 1.3351x over previous
"""Causal self-attention kernel for Trainium2, 8-core data parallel.

Per-core program: one batch element b of x [8, 1024, 768].
  - x, W_attn, W_proj transposed into c-on-partitions layouts via PE
    identity-matmul transposes (fp32r: 1.5 cyc/row).
  - V projection [t, j] first; Q/K projections per head-pair interleaved
    into the attention loop so ACT-engine exp work hides under PE
    projection matmuls.
  - per head pair: S^T = K^T.T @ Q^T with 64-row PE tiling (even head in
    rows 0-63, odd head in rows 64-127, concurrent); P^T = exp(S^T/8)
    with causal tri-mask on diagonal blocks (gpsimd affine_select);
    y^T_h = V-aug slices @ P^T with a ones column accumulating softmax
    denominators.
  - normalization off the PE critical path: reciprocal_approx_fast (DVE)
    + DMA partition-broadcast + one DVE multiply per head.
  - out = y @ Wp^T + b_p (t on partitions).
All matmuls bf16 inputs / fp32 PSUM accumulation; softmax in fp32.
"""
import sys
import contextlib
from contextlib import ExitStack

sys.path.insert(0, "/opt/trn_rl_repo")

import numpy as np

import concourse.bass as bass
import concourse.bacc as bacc
import concourse.mybir as mybir
import concourse.tile as tile
from concourse.masks import make_upper_triangular

F32 = mybir.dt.float32
F32R = mybir.dt.float32r
BF16 = mybir.dt.bfloat16
P = 128
T = 1024
C = 768
H = 12
HS = 64
CT = C // P     # 6 c-tiles
TT = T // P     # 8 t-tiles
KT = T // P     # 8 k-tiles per head
N_CORES = 8
NORM_RECIP_FAST = True
NORM_DMA_BCAST = True


def qk_chunks(kt):
    """[(q0, w)] matmul chunks for k-tile kt: q >= kt*128, each split so no
    chunk exceeds 512 (one PSUM bank) or straddles a 512 boundary."""
    qs = kt * P
    if kt < 4:
        return [(qs, 512 - qs), (512, 512)]
    w = T - qs
    h0 = (w // 2) // 64 * 64
    return [(qs, h0), (qs + h0, w - h0)]


def emit_consts(nc, tc, const, ba_d, bp_d):
    from concourse.masks import make_identity
    ident = const.tile([P, P], F32, tag="ident")
    make_identity(nc, ident)
    tri = const.tile([P, P], BF16, tag="tri")
    make_upper_triangular(nc, tri, val=1.0, diag=True)
    ones_bf = const.tile([P, P], BF16, tag="ones")
    nc.gpsimd.memset(ones_bf[:], 1.0)
    bias_qk = const.tile([P, 12], F32, tag="bqk")
    nc.sync.dma_start(bias_qk[:], ba_d[0 : 2 * C].rearrange("(o p) -> p o", p=P))
    Bv = const.tile([P, C], F32, tag="Bv")
    nc.sync.dma_start(
        Bv[:],
        ba_d[2 * C : 3 * C].rearrange("(a j) -> a j", a=1).to_broadcast([P, C]),
    )
    Bp = const.tile([P, C], F32, tag="Bp")
    nc.sync.dma_start(
        Bp[:], bp_d.rearrange("(a j) -> a j", a=1).to_broadcast([P, C])
    )
    return dict(ident=ident, tri=tri, ones_bf=ones_bf, bias_qk=bias_qk, Bv=Bv, Bp=Bp)


def emit_body(nc, tc, pools, cst, x_d, wa_d, wp_d, y_d):
    const, persist, nat, work, ptp, dram, psA, psB = pools
    bias_qk, Bv, Bp = cst["bias_qk"], cst["Bv"], cst["Bp"]
    cst_ones = cst["ones_bf"]
    ident = cst["ident"]

    xT = persist.tile([P, CT, T], BF16, tag="xT")
    WT = persist.tile([P, CT, 3 * C], BF16, tag="WT")
    WpT = persist.tile([P, CT, C], BF16, tag="WpT")
    qT = persist.tile([P, 6, T], BF16, tag="qT")     # Q features, pair layout
    KTp = persist.tile([P, 6, T], BF16, tag="KTp")   # K features, pair layout
    V = persist.tile([P, TT, 12, 65], BF16, tag="Vaug")
    yT = persist.tile([P, CT, T], BF16, tag="yT")

    # ---- phase 0: transpose x, W_attn, W_proj via PE (identity matmul)
    def transpose_in(dram_src, n_row_tiles, dst):
        src = dram_src.rearrange("(rt p) c -> rt p c", p=P)
        for rt in range(n_row_tiles):
            natt = nat.tile([P, C], F32, tag="nat")
            nc.sync.dma_start(natt[:], src[rt])
            for g in range(2):
                ps = psA.tile([P, 512], F32, tag="mm", name="tps")[:, :384]
                for i in range(3):
                    ct = g * 3 + i
                    nc.tensor.transpose(
                        ps[:, i * P : (i + 1) * P],
                        natt[:, ct * P : (ct + 1) * P],
                        ident[:],
                    )
                eng = nc.vector if rt % 2 == 0 else nc.scalar
                dstap = dst[:, g * 3 : (g + 1) * 3, rt * P : (rt + 1) * P]
                srcap = ps[:].rearrange("p (a b) -> p a b", a=3)
                if eng is nc.vector:
                    nc.vector.tensor_copy(dstap, srcap)
                else:
                    nc.scalar.activation(
                        dstap, srcap, mybir.ActivationFunctionType.Copy
                    )

    transpose_in(x_d, TT, xT)
    transpose_in(wa_d, 18, WT)
    transpose_in(wp_d, CT, WpT)

    # ---- phase 1a: V projection, [t, j] layout (stationary xT tiles)
    for tt in range(TT):
        pss = [psA.tile([P, 512], F32, tag="mm", name=f"vps{i}")[:, :384] for i in range(2)]
        for ct in range(CT):
            for jb in range(2):
                nc.tensor.matmul(
                    pss[jb][:],
                    xT[:, ct, tt * P : (tt + 1) * P],
                    WT[:, ct, 2 * C + jb * 384 : 2 * C + (jb + 1) * 384],
                    start=(ct == 0),
                    stop=(ct == CT - 1),
                )
        for jb in range(2):
            sl = slice(jb * 384, (jb + 1) * 384)
            nc.vector.tensor_add(
                V[:, tt, 6 * jb : 6 * jb + 6, 0:64],
                pss[jb][:].rearrange("p (h d) -> p h d", d=64),
                Bv[:, sl].rearrange("p (h d) -> p h d", d=64),
            )

    # ---- phase 1b helper: Q/K projection for head pair j (jt = j and 6+j)
    def emit_qkproj_pair(j):
        for jt in (j, 6 + j):
            pss = [psA.tile([P, 512], F32, tag="mm", name=f"qkps{i}") for i in range(2)]
            for ct in range(CT):
                for tb in range(2):
                    nc.tensor.matmul(
                        pss[tb][:],
                        WT[:, ct, jt * P : (jt + 1) * P],
                        xT[:, ct, tb * 512 : (tb + 1) * 512],
                        start=(ct == 0),
                        stop=(ct == CT - 1),
                    )
            for tb in range(2):
                sl = slice(tb * 512, (tb + 1) * 512)
                if jt < 6:
                    nc.scalar.activation(
                        qT[:, j, sl], pss[tb][:],
                        mybir.ActivationFunctionType.Identity,
                        bias=bias_qk[:, jt : jt + 1],
                    )
                else:
                    nc.vector.tensor_scalar_add(
                        KTp[:, j, sl], pss[tb][:], bias_qk[:, jt : jt + 1]
                    )

    # ---- phase 2 helpers
    PTs = {}

    def emit_qk_pair(j):
        """S^T and P^T for heads 2j (PE rows 0-63) and 2j+1 (rows 64-127),
        row-tiled to run concurrently on the PE."""
        PTa = ptp.tile([P, KT, T], BF16, tag="PT", name=f"PTa{j}")
        PTb = ptp.tile([P, KT, T], BF16, tag="PT", name=f"PTb{j}")
        PTs[2 * j] = PTa
        PTs[2 * j + 1] = PTb
        for kt in range(KT):
            qs = kt * P
            for (q0, w) in qk_chunks(kt):
                spsa = psA.tile([P, 512], F32, tag="mm", name="spsa")
                spsb = psA.tile([P, 512], F32, tag="mm", name="spsb")
                nc.tensor.matmul(
                    spsa[:, :w],
                    KTp[0:64, j, kt * P : (kt + 1) * P],
                    qT[0:64, j, q0 : q0 + w],
                    start=True, stop=True,
                    tile_position=(0, 0),
                )
                nc.tensor.matmul(
                    spsb[:, :w],
                    KTp[64:128, j, kt * P : (kt + 1) * P],
                    qT[64:128, j, q0 : q0 + w],
                    start=True, stop=True,
                    tile_position=(64, 0),
                )
                nc.scalar.activation(
                    PTa[:, kt, q0 : q0 + w], spsa[:, :w],
                    mybir.ActivationFunctionType.Exp, scale=0.125,
                )
                nc.scalar.activation(
                    PTb[:, kt, q0 : q0 + w], spsb[:, :w],
                    mybir.ActivationFunctionType.Exp, scale=0.125,
                )
            for PT in (PTa, PTb):
                nc.gpsimd.affine_select(
                    out=PT[:, kt, qs : qs + P],
                    in_=PT[:, kt, qs : qs + P],
                    compare_op=mybir.AluOpType.is_ge,
                    fill=0.0,
                    base=0,
                    # keep where (-k + q') >= 0, else fill 0
                    pattern=[[1, P]],
                    channel_multiplier=-1,
                )

    def emit_pv_head(h):
        PT = PTs.pop(h)
        yD = psB.tile([P, 1024], F32, tag="yD", name=f"yD{h}")
        for b in range(2):
            for kt in range(KT):
                if b == 0 and kt >= 4:
                    continue
                off = max(0, kt * P - b * 512)
                w = 512 - off
                first = kt == 0
                last = kt == (3 if b == 0 else KT - 1)
                nc.tensor.matmul(
                    yD[0:65, b * 512 + off : b * 512 + off + w],
                    V[:, kt, h, :],
                    PT[:, kt, b * 512 + off : b * 512 + off + w],
                    start=first,
                    stop=last,
                )
        return yD

    def emit_norm_head(h, yD):
        """yT_h = yD[0:64] * (1/D) with D = yD[64]; no PE involvement."""
        Rh = work.tile([64, T], F32, tag="Rh")
        if NORM_DMA_BCAST:
            # DMA D row to DRAM, broadcast-read to 64 partitions, then
            # reciprocal at base partition 0 (approx_fast needs base 0).
            Dsb = work.tile([65, T], F32, tag="Dsb")
            nc.scalar.activation(
                Dsb[64:65, :], yD[64:65, :], mybir.ActivationFunctionType.Copy
            )
            Dd = dram.tile([T], F32, tag="Dd")
            nc.sync.dma_start(Dd[:].rearrange("(a t) -> a t", a=1), Dsb[64:65, :])
            Dfull = work.tile([64, T], F32, tag="Dfull")
            nc.sync.dma_start(
                Dfull[0:64, :],
                Dd[:].rearrange("(a t) -> a t", a=1).to_broadcast([64, T]),
            )
            if NORM_RECIP_FAST:
                nc.vector.reciprocal_approx_fast(Rh[0:64, :], Dfull[0:64, :])
            else:
                nc.vector.reciprocal(Rh[0:64, :], Dfull[0:64, :])
        else:
            Dr = work.tile([65, T], F32, tag="Dr")
            nc.vector.reciprocal(Dr[64:65, :], yD[64:65, :])
            # PE broadcast: ones column at tile_position (64, 0)
            Drb = work.tile([65, T], BF16, tag="Drb")
            nc.vector.tensor_copy(Drb[64:65, :], Dr[64:65, :])
            for b in range(2):
                rps = psA.tile([P, 512], F32, tag="mm", name="rps")
                nc.tensor.matmul(
                    rps[0:64, :],
                    cst_ones[64:65, 0:64],
                    Drb[64:65, b * 512 : (b + 1) * 512],
                    start=True, stop=True,
                    tile_position=(64, 0),
                )
                nc.scalar.activation(
                    Rh[0:64, b * 512 : (b + 1) * 512], rps[0:64, :],
                    mybir.ActivationFunctionType.Copy,
                )
        if h % 2 == 0:
            nc.vector.tensor_mul(yT[0:64, h // 2, :], yD[0:64, :], Rh[0:64, :])
        else:
            yTt = work.tile([64, T], BF16, tag="yTt")
            nc.vector.tensor_mul(yTt[0:64, :], yD[0:64, :], Rh[0:64, :])
            nc.sync.dma_start(yT[64:128, h // 2, :], yTt[0:64, :])

    # ---- phase 1b + 2 interleaved: per pair, QK matmuls come first, then the
    # next pair's Q/K projections (PE work hiding the exps), then PV + norm.
    emit_qkproj_pair(0)
    for p in range(6):
        emit_qk_pair(p)
        if p + 1 < 6:
            emit_qkproj_pair(p + 1)
        yD0 = emit_pv_head(2 * p)
        emit_norm_head(2 * p, yD0)
        yD1 = emit_pv_head(2 * p + 1)
        emit_norm_head(2 * p + 1, yD1)

    # ---- phase 3: out = y @ Wp^T + b_p
    for tt in range(TT):
        osb = work.tile([P, C], F32, tag="osb")
        pss = [psA.tile([P, 512], F32, tag="mm", name=f"ops{i}")[:, :384] for i in range(2)]
        for ct in range(CT):
            for jb in range(2):
                nc.tensor.matmul(
                    pss[jb][:],
                    yT[:, ct, tt * P : (tt + 1) * P],
                    WpT[:, ct, jb * 384 : (jb + 1) * 384],
                    start=(ct == 0),
                    stop=(ct == CT - 1),
                )
        for jb in range(2):
            sl = slice(jb * 384, (jb + 1) * 384)
            nc.vector.tensor_add(osb[:, sl], pss[jb][:], Bp[:, sl])
        nc.sync.dma_start(
            y_d.rearrange("(tt p) c -> tt p c", p=P)[tt], osb[:]
        )


def build_program(loop=1):
    nc = bacc.Bacc("TRN2", target_bir_lowering=False, debug=False)
    x_d = nc.dram_tensor("x", [T, C], F32, kind="ExternalInput").ap()
    wa_d = nc.dram_tensor("W_attn", [3 * C, C], F32, kind="ExternalInput").ap()
    ba_d = nc.dram_tensor("b_attn", [3 * C], F32, kind="ExternalInput").ap()
    wp_d = nc.dram_tensor("W_proj", [C, C], F32, kind="ExternalInput").ap()
    bp_d = nc.dram_tensor("b_proj", [C], F32, kind="ExternalInput").ap()
    y_d = nc.dram_tensor("y", [T, C], F32, kind="ExternalOutput").ap()

    with tile.TileContext(nc) as tc, ExitStack() as ctx:
        const = ctx.enter_context(tc.tile_pool(name="const", bufs=1))
        persist = ctx.enter_context(tc.tile_pool(name="persist", bufs=1))
        nat = ctx.enter_context(tc.tile_pool(name="nat", bufs=6))
        work = ctx.enter_context(tc.tile_pool(name="work", bufs=2))
        ptp = ctx.enter_context(tc.tile_pool(name="ptp", bufs=3))
        dram = ctx.enter_context(tc.tile_pool(name="dram", bufs=2, space="DRAM"))
        psA = ctx.enter_context(tc.tile_pool(name="psA", bufs=4, space="PSUM"))
        psB = ctx.enter_context(tc.tile_pool(name="psB", bufs=2, space="PSUM"))
        pools = (const, persist, nat, work, ptp, dram, psA, psB)

        cst = emit_consts(nc, tc, const, ba_d, bp_d)
        # V ones column (softmax denominator accumulator) set once
        V0 = persist.tile([P, TT, 12, 65], BF16, tag="Vaug")
        nc.gpsimd.memset(V0[:, :, :, 64:65], 1.0)
        loop_cm = tc.For_i(0, loop, 1) if loop > 1 else contextlib.nullcontext()
        with loop_cm:
            emit_body(nc, tc, pools, cst, x_d, wa_d, wp_d, y_d)

    nc.compile()
    return nc


_CACHED_NC = None


def kernel(x, W_attn, b_attn, W_proj, b_proj):
    from concourse.bass_utils import run_bass_kernel_spmd

    global _CACHED_NC
    if _CACHED_NC is None:
        _CACHED_NC = build_program(loop=1)
    nc = _CACHED_NC

    B = x.shape[0]
    assert B == N_CORES
    in_maps = [
        {
            "x": np.ascontiguousarray(x[b], dtype=np.float32),
            "W_attn": np.asarray(W_attn, dtype=np.float32),
            "b_attn": np.asarray(b_attn, dtype=np.float32),
            "W_proj": np.asarray(W_proj, dtype=np.float32),
            "b_proj": np.asarray(b_proj, dtype=np.float32),
        }
        for b in range(B)
    ]
    res = run_bass_kernel_spmd(nc, in_maps, list(range(N_CORES)))
    return np.stack([res.results[b]["y"] for b in range(B)], axis=0)


# revision 18
# speedup vs baseline: 1.5704x; 1.1763x over previous
"""Causal self-attention kernel for Trainium2, 8-core data parallel.

Per-core program: one batch element b of x [8, 1024, 768].
  - x, W_attn, W_proj transposed into c-on-partitions layouts via PE
    identity-matmul transposes.
  - V projection [t, j] first; Q/K projections for pair p+1 interleaved
    instruction-by-instruction with pair p's QK matmuls so the PE stays
    busy while the ACT engine runs the softmax exps.
  - per head pair: S^T = K^T.T @ Q^T with 64-row PE tiling (even head in
    rows 0-63, odd head in rows 64-127, concurrent); one exp per k-tile
    over a [128, 1024] PSUM tile; causal tri-mask on diagonal blocks
    (gpsimd affine_select); y^T_h = V-aug slices @ P^T with a ones
    column accumulating softmax denominators.
  - normalization off the PE critical path: D row -> DRAM -> broadcast
    read to 64 partitions -> reciprocal_approx_fast -> one DVE multiply.
  - out = y @ Wp^T + b_p (t on partitions).
All matmuls bf16 inputs / fp32 PSUM accumulation; softmax in fp32.
"""
import sys
import contextlib
from contextlib import ExitStack

sys.path.insert(0, "/opt/trn_rl_repo")

import numpy as np

import concourse.bass as bass
import concourse.bacc as bacc
import concourse.mybir as mybir
import concourse.tile as tile

F32 = mybir.dt.float32
BF16 = mybir.dt.bfloat16
P = 128
T = 1024
C = 768
H = 12
HS = 64
CT = C // P     # 6 c-tiles
TT = T // P     # 8 t-tiles
KT = T // P     # 8 k-tiles per head
N_CORES = 8


def qk_chunks(kt):
    """[(q0, w)] matmul chunks for k-tile kt covering q in [kt*128, 1024),
    each within one PSUM bank (512 fp32, 512-aligned)."""
    qs = kt * P
    if kt < 4:
        return [(qs, 512 - qs), (512, 512)]
    return [(qs, T - qs)]


def emit_consts(nc, tc, const, ba_d, bp_d):
    from concourse.masks import make_identity
    ident = const.tile([P, P], F32, tag="ident")
    make_identity(nc, ident)
    bias_qk = const.tile([P, 12], F32, tag="bqk")
    nc.sync.dma_start(bias_qk[:], ba_d[0 : 2 * C].rearrange("(o p) -> p o", p=P))
    Bv = const.tile([P, C], F32, tag="Bv")
    nc.sync.dma_start(
        Bv[:],
        ba_d[2 * C : 3 * C].rearrange("(a j) -> a j", a=1).to_broadcast([P, C]),
    )
    Bp = const.tile([P, C], F32, tag="Bp")
    nc.sync.dma_start(
        Bp[:], bp_d.rearrange("(a j) -> a j", a=1).to_broadcast([P, C])
    )
    return dict(ident=ident, bias_qk=bias_qk, Bv=Bv, Bp=Bp)


def emit_body(nc, tc, pools, cst, x_d, wa_d, wp_d, y_d):
    const, persist, nat, work, ptp, dram, psS, psP = pools
    bias_qk, Bv, Bp = cst["bias_qk"], cst["Bv"], cst["Bp"]
    ident = cst["ident"]

    xT = persist.tile([P, CT, T], BF16, tag="xT")
    WT = persist.tile([P, CT, 3 * C], BF16, tag="WT")
    WpT = persist.tile([P, CT, C], BF16, tag="WpT")
    qT = persist.tile([P, 6, T], BF16, tag="qT")     # Q features, pair layout
    KTp = persist.tile([P, 6, T], BF16, tag="KTp")   # K features, pair layout
    V = persist.tile([P, TT, 12, 65], BF16, tag="Vaug")
    yT = persist.tile([P, CT, T], BF16, tag="yT")

    # ---- phase 0: transpose tiles via PE; one [128,1024] psum tile per
    # row-tile, halves at [0:384] (bank 0) and [512:896] (bank 1).
    def transpose_tile(dram_src_rt, dst, rt):
        natt = nat.tile([P, C], F32, tag="nat")
        nc.sync.dma_start(natt[:], dram_src_rt)
        ps = psP.tile([P, 1024], F32, tag="pp", name="tps")
        for g in range(2):
            base = g * 512
            for i in range(3):
                ct = g * 3 + i
                nc.tensor.transpose(
                    ps[:, base + i * P : base + (i + 1) * P],
                    natt[:, ct * P : (ct + 1) * P],
                    ident[:],
                )
        for g in range(2):
            dstap = dst[:, g * 3 : (g + 1) * 3, rt * P : (rt + 1) * P]
            srcap = ps[:, g * 512 : g * 512 + 384].rearrange("p (a b) -> p a b", a=3)
            if rt % 2 == 0:
                nc.vector.tensor_copy(dstap, srcap)
            else:
                nc.scalar.activation(
                    dstap, srcap, mybir.ActivationFunctionType.Copy
                )

    def transpose_in(dram_src, n_row_tiles, dst):
        src = dram_src.rearrange("(rt p) c -> rt p c", p=P)
        for rt in range(n_row_tiles):
            transpose_tile(src[rt], dst, rt)

    transpose_in(x_d, TT, xT)
    transpose_in(wa_d, 18, WT)
    # W_proj transposes deferred into the last attention pair's stretch.

    # ---- phase 1a: V projection, [t, j] layout (stationary xT tiles)
    for tt in range(TT):
        pss = psP.tile([P, 1024], F32, tag="pp", name="vps")
        for jb in range(2):
            for ct in range(CT):
                nc.tensor.matmul(
                    pss[:, jb * 512 : jb * 512 + 384],
                    xT[:, ct, tt * P : (tt + 1) * P],
                    WT[:, ct, 2 * C + jb * 384 : 2 * C + (jb + 1) * 384],
                    start=(ct == 0),
                    stop=(ct == CT - 1),
                )
        for jb in range(2):
            sl = slice(jb * 384, (jb + 1) * 384)
            nc.vector.tensor_add(
                V[:, tt, 6 * jb : 6 * jb + 6, 0:64],
                pss[:, jb * 512 : jb * 512 + 384].rearrange("p (h d) -> p h d", d=64),
                Bv[:, sl].rearrange("p (h d) -> p h d", d=64),
            )

    # ---- phase 1b: Q/K projection for head pair j as a list of MM thunks
    # (interleaved into the attention stream) + a finish step (DVE copies).
    def qkproj_alloc(j):
        pQ = psP.tile([P, 1024], F32, tag="pp", name=f"pQ{j}")
        pK = psP.tile([P, 1024], F32, tag="pp", name=f"pK{j}")
        return (pQ, pK)

    def qkproj_mms(j, tiles):
        pQ, pK = tiles
        thunks = []
        for jt, pt in ((j, pQ), (6 + j, pK)):
            for tb in range(2):
                for ct in range(CT):
                    def mm(jt=jt, pt=pt, tb=tb, ct=ct):
                        nc.tensor.matmul(
                            pt[:, tb * 512 : (tb + 1) * 512],
                            WT[:, ct, jt * P : (jt + 1) * P],
                            xT[:, ct, tb * 512 : (tb + 1) * 512],
                            start=(ct == 0),
                            stop=(ct == CT - 1),
                        )
                    thunks.append(mm)
        return thunks

    def qkproj_finish(j, tiles):
        pQ, pK = tiles
        nc.vector.tensor_scalar_add(qT[:, j, :], pQ[:], bias_qk[:, j : j + 1])
        nc.vector.tensor_scalar_add(KTp[:, j, :], pK[:], bias_qk[:, 6 + j : 7 + j])

    # ---- phase 2 helpers
    def emit_qk_pair(j, filler):
        """S^T and P^T for heads 2j (PE rows 0-63) and 2j+1 (rows 64-127),
        row-tiled concurrent; `filler` thunks (dense PE work for pair j+1)
        are woven between k-tiles to keep the PE busy during exps."""
        PTa = ptp.tile([P, KT, T], BF16, tag="PT", name=f"PTa{j}")
        PTb = ptp.tile([P, KT, T], BF16, tag="PT", name=f"PTb{j}")
        fi = 0
        nf = len(filler)
        for kt in range(KT):
            qs = kt * P
            spsa = psS.tile([P, 1024], F32, tag="sps", name="spsa")
            spsb = psS.tile([P, 1024], F32, tag="sps", name="spsb")
            for (q0, w) in qk_chunks(kt):
                nc.tensor.matmul(
                    spsa[:, q0 : q0 + w],
                    KTp[0:64, j, kt * P : (kt + 1) * P],
                    qT[0:64, j, q0 : q0 + w],
                    start=True, stop=True,
                    tile_position=(0, 0),
                )
                nc.tensor.matmul(
                    spsb[:, q0 : q0 + w],
                    KTp[64:128, j, kt * P : (kt + 1) * P],
                    qT[64:128, j, q0 : q0 + w],
                    start=True, stop=True,
                    tile_position=(64, 0),
                )
            nc.scalar.activation(
                PTa[:, kt, qs:], spsa[:, qs:],
                mybir.ActivationFunctionType.Exp, scale=0.125,
            )
            nc.scalar.activation(
                PTb[:, kt, qs:], spsb[:, qs:],
                mybir.ActivationFunctionType.Exp, scale=0.125,
            )
            for PT in (PTa, PTb):
                nc.gpsimd.affine_select(
                    out=PT[:, kt, qs : qs + P],
                    in_=PT[:, kt, qs : qs + P],
                    compare_op=mybir.AluOpType.is_ge,
                    fill=0.0,
                    base=0,
                    # keep where (-k + q') >= 0, else fill 0
                    pattern=[[1, P]],
                    channel_multiplier=-1,
                )
            # weave in dense filler MMs for the next pair
            take = (nf * (kt + 1)) // KT - fi
            for _ in range(take):
                filler[fi]()
                fi += 1
        while fi < nf:
            filler[fi]()
            fi += 1
        return PTa, PTb

    def emit_pv_head(h, PT, yD, blocks=(0, 1)):
        for b in blocks:
            for kt in range(KT):
                if b == 0 and kt >= 4:
                    continue
                off = max(0, kt * P - b * 512)
                w = 512 - off
                first = kt == 0
                last = kt == (3 if b == 0 else KT - 1)
                nc.tensor.matmul(
                    yD[0:65, b * 512 + off : b * 512 + off + w],
                    V[:, kt, h, :],
                    PT[:, kt, b * 512 + off : b * 512 + off + w],
                    start=first,
                    stop=last,
                )

    def emit_norm_head(h, yD, b0=0, b1=2):
        """yT_h[b0*512:(b1)*512] = yD[0:64] * (1/D), D = yD[64]; PE-free."""
        n = (b1 - b0) * 512
        sl = slice(b0 * 512, b1 * 512)
        Dsb = work.tile([65, T], F32, tag="Dsb")
        nc.vector.tensor_copy(Dsb[64:65, sl], yD[64:65, sl])
        Dd = dram.tile([T], F32, tag="Dd")
        nc.sync.dma_start(
            Dd[sl].rearrange("(a t) -> a t", a=1), Dsb[64:65, sl]
        )
        Dfull = work.tile([64, T], F32, tag="Dfull")
        nc.sync.dma_start(
            Dfull[0:64, sl],
            Dd[sl].rearrange("(a t) -> a t", a=1).to_broadcast([64, n]),
        )
        Rh = work.tile([64, T], F32, tag="Rh")
        nc.vector.reciprocal_approx_fast(Rh[0:64, sl], Dfull[0:64, sl])
        if h % 2 == 0:
            nc.vector.tensor_mul(yT[0:64, h // 2, sl], yD[0:64, sl], Rh[0:64, sl])
        else:
            yTt = work.tile([64, T], BF16, tag="yTt")
            nc.vector.tensor_mul(yTt[0:64, sl], yD[0:64, sl], Rh[0:64, sl])
            nc.sync.dma_start(yT[64:128, h // 2, sl], yTt[0:64, sl])

    def emit_outproj(tt):
        pss = psP.tile([P, 1024], F32, tag="pp", name="ops")
        for jb in range(2):
            for ct in range(CT):
                nc.tensor.matmul(
                    pss[:, jb * 512 : jb * 512 + 384],
                    yT[:, ct, tt * P : (tt + 1) * P],
                    WpT[:, ct, jb * 384 : (jb + 1) * 384],
                    start=(ct == 0),
                    stop=(ct == CT - 1),
                )
        osb = work.tile([P, C], F32, tag="osb")
        for jb in range(2):
            sl = slice(jb * 384, (jb + 1) * 384)
            nc.vector.tensor_add(osb[:, sl], pss[:, jb * 512 : jb * 512 + 384], Bp[:, sl])
        nc.sync.dma_start(y_d.rearrange("(tt p) c -> tt p c", p=P)[tt], osb[:])

    # ---- phase 1b + 2 interleaved pair loop
    tiles0 = qkproj_alloc(0)
    for mm in qkproj_mms(0, tiles0):
        mm()
    qkproj_finish(0, tiles0)
    tiles = tiles0
    wp_src = wp_d.rearrange("(rt p) c -> rt p c", p=P)
    for p in range(6):
        if p + 1 < 6:
            nxt = qkproj_alloc(p + 1)
            filler = qkproj_mms(p + 1, nxt)
        else:
            nxt = None
            filler = [
                (lambda rt=rt: transpose_tile(wp_src[rt], WpT, rt))
                for rt in range(CT)
            ]
        PTa, PTb = emit_qk_pair(p, filler)
        if nxt is not None:
            qkproj_finish(p + 1, nxt)
        if p < 5:
            yD0 = psP.tile([P, 1024], F32, tag="pp", name=f"yD{2*p}")
            emit_pv_head(2 * p, PTa, yD0)
            emit_norm_head(2 * p, yD0)
            yD1 = psP.tile([P, 1024], F32, tag="pp", name=f"yD{2*p+1}")
            emit_pv_head(2 * p + 1, PTb, yD1)
            emit_norm_head(2 * p + 1, yD1)
        else:
            # last pair: split by q-block halves so out-proj tt 0-3 can
            # start while the b=1 normalization chain drains.
            yD0 = psP.tile([P, 1024], F32, tag="pp", name="yD10")
            yD1 = psP.tile([P, 1024], F32, tag="pp", name="yD11")
            emit_pv_head(10, PTa, yD0, blocks=(0,))
            emit_norm_head(10, yD0, 0, 1)
            emit_pv_head(11, PTb, yD1, blocks=(0,))
            emit_norm_head(11, yD1, 0, 1)
            emit_pv_head(10, PTa, yD0, blocks=(1,))
            emit_norm_head(10, yD0, 1, 2)
            emit_pv_head(11, PTb, yD1, blocks=(1,))
            emit_norm_head(11, yD1, 1, 2)
        tiles = nxt

    # ---- phase 3: out = y @ Wp^T + b_p
    for tt in range(TT):
        emit_outproj(tt)


def build_program(loop=1):
    nc = bacc.Bacc("TRN2", target_bir_lowering=False, debug=False)
    x_d = nc.dram_tensor("x", [T, C], F32, kind="ExternalInput").ap()
    wa_d = nc.dram_tensor("W_attn", [3 * C, C], F32, kind="ExternalInput").ap()
    ba_d = nc.dram_tensor("b_attn", [3 * C], F32, kind="ExternalInput").ap()
    wp_d = nc.dram_tensor("W_proj", [C, C], F32, kind="ExternalInput").ap()
    bp_d = nc.dram_tensor("b_proj", [C], F32, kind="ExternalInput").ap()
    y_d = nc.dram_tensor("y", [T, C], F32, kind="ExternalOutput").ap()

    with tile.TileContext(nc) as tc, ExitStack() as ctx:
        const = ctx.enter_context(tc.tile_pool(name="const", bufs=1))
        persist = ctx.enter_context(tc.tile_pool(name="persist", bufs=1))
        nat = ctx.enter_context(tc.tile_pool(name="nat", bufs=6))
        work = ctx.enter_context(tc.tile_pool(name="work", bufs=2))
        ptp = ctx.enter_context(tc.tile_pool(name="ptp", bufs=3))
        dram = ctx.enter_context(tc.tile_pool(name="dram", bufs=2, space="DRAM"))
        psS = ctx.enter_context(tc.tile_pool(name="psS", bufs=2, space="PSUM"))
        psP = ctx.enter_context(tc.tile_pool(name="psP", bufs=2, space="PSUM"))
        pools = (const, persist, nat, work, ptp, dram, psS, psP)

        cst = emit_consts(nc, tc, const, ba_d, bp_d)
        # V ones column (softmax denominator accumulator) set once
        V0 = persist.tile([P, TT, 12, 65], BF16, tag="Vaug")
        nc.gpsimd.memset(V0[:, :, :, 64:65], 1.0)
        loop_cm = tc.For_i(0, loop, 1) if loop > 1 else contextlib.nullcontext()
        with loop_cm:
            emit_body(nc, tc, pools, cst, x_d, wa_d, wp_d, y_d)

    nc.compile()
    return nc


_CACHED_NC = None


def kernel(x, W_attn, b_attn, W_proj, b_proj):
    from concourse.bass_utils import run_bass_kernel_spmd

    global _CACHED_NC
    if _CACHED_NC is None:
        _CACHED_NC = build_program(loop=1)
    nc = _CACHED_NC

    B = x.shape[0]
    assert B == N_CORES
    in_maps = [
        {
            "x": np.ascontiguousarray(x[b], dtype=np.float32),
            "W_attn": np.asarray(W_attn, dtype=np.float32),
            "b_attn": np.asarray(b_attn, dtype=np.float32),
            "W_proj": np.asarray(W_proj, dtype=np.float32),
            "b_proj": np.asarray(b_proj, dtype=np.float32),
        }
        for b in range(B)
    ]
    res = run_bass_kernel_spmd(nc, in_maps, list(range(N_CORES)))
    return np.stack([res.results[b]["y"] for b in range(B)], axis=0)


# revision 22
# speedup vs baseline: 1.5898x; 1.0124x over previous
"""Causal self-attention kernel for Trainium2, 8-core data parallel.

Per-core program: one batch element b of x [8, 1024, 768].
  - x, W_attn, W_proj transposed into c-on-partitions layouts via PE
    identity-matmul transposes; V projection interleaved with the Q/K
    weight transposes so PE work covers the DMA-bound load phase.
  - Q/K projections for pair p+1 interleaved instruction-by-instruction
    with pair p's QK matmuls so the PE stays busy while the ACT engine
    runs the softmax exps.
  - per head pair: S^T = K^T.T @ Q^T with 64-row PE tiling (even head in
    rows 0-63, odd head in rows 64-127, concurrent); exp per PSUM-bank
    chunk (ring of 4 so exps pipeline); causal tri-mask on diagonal
    blocks (gpsimd affine_select); y^T_h = V-aug slices @ P^T with a
    ones column accumulating softmax denominators.
  - normalization off the PE critical path: D row -> DRAM -> broadcast
    read to 64 partitions -> reciprocal_approx_fast -> one DVE multiply.
  - out = y @ Wp^T + b_p (t on partitions).
All matmuls bf16 inputs / fp32 PSUM accumulation; softmax in fp32.
"""
import sys
import contextlib
from contextlib import ExitStack

sys.path.insert(0, "/opt/trn_rl_repo")

import numpy as np

import concourse.bass as bass
import concourse.bacc as bacc
import concourse.mybir as mybir
import concourse.tile as tile

F32 = mybir.dt.float32
BF16 = mybir.dt.bfloat16
P = 128
T = 1024
C = 768
H = 12
HS = 64
CT = C // P     # 6 c-tiles
TT = T // P     # 8 t-tiles
KT = T // P     # 8 k-tiles per head
N_CORES = 8


def qk_chunks(kt):
    """[(q0, w)] matmul chunks for k-tile kt covering q in [kt*128, 1024),
    each within one PSUM bank (<=512 fp32, 512-aligned)."""
    qs = kt * P
    if kt < 4:
        return [(qs, 512 - qs), (512, 512)]
    return [(qs, T - qs)]


def emit_consts(nc, tc, const, ba_d, bp_d):
    from concourse.masks import make_identity
    ident = const.tile([P, P], F32, tag="ident")
    make_identity(nc, ident)
    bias_qk = const.tile([P, 12], F32, tag="bqk")
    nc.sync.dma_start(bias_qk[:], ba_d[0 : 2 * C].rearrange("(o p) -> p o", p=P))
    Bv = const.tile([P, C], F32, tag="Bv")
    nc.sync.dma_start(
        Bv[:],
        ba_d[2 * C : 3 * C].rearrange("(a j) -> a j", a=1).to_broadcast([P, C]),
    )
    Bp = const.tile([P, C], F32, tag="Bp")
    nc.sync.dma_start(
        Bp[:], bp_d.rearrange("(a j) -> a j", a=1).to_broadcast([P, C])
    )
    return dict(ident=ident, bias_qk=bias_qk, Bv=Bv, Bp=Bp)


def emit_body(nc, tc, pools, cst, x_d, wa_d, wp_d, y_d):
    const, persist, nat, work, ptp, dram, psS, psP = pools
    bias_qk, Bv, Bp = cst["bias_qk"], cst["Bv"], cst["Bp"]
    ident = cst["ident"]

    xT = persist.tile([P, CT, T], BF16, tag="xT")
    WT = persist.tile([P, CT, 3 * C], BF16, tag="WT")
    WpT = persist.tile([P, CT, C], BF16, tag="WpT")
    qT = persist.tile([P, 6, T], BF16, tag="qT")     # Q features, pair layout
    KTp = persist.tile([P, 6, T], BF16, tag="KTp")   # K features, pair layout
    V = persist.tile([P, TT, 12, 65], BF16, tag="Vaug")
    yT = persist.tile([P, CT, T], BF16, tag="yT")

    # ---- transposes via PE; one [128,1024] psum tile per row-tile,
    # halves at [0:384] (bank 0) and [512:896] (bank 1).
    def transpose_tile(dram_src_rt, dst, rt):
        natt = nat.tile([P, C], F32, tag="nat")
        nc.sync.dma_start(natt[:], dram_src_rt)
        ps = psP.tile([P, 1024], F32, tag="pp", name="tps")
        for g in range(2):
            base = g * 512
            for i in range(3):
                ct = g * 3 + i
                nc.tensor.transpose(
                    ps[:, base + i * P : base + (i + 1) * P],
                    natt[:, ct * P : (ct + 1) * P],
                    ident[:],
                )
        for g in range(2):
            dstap = dst[:, g * 3 : (g + 1) * 3, rt * P : (rt + 1) * P]
            srcap = ps[:, g * 512 : g * 512 + 384].rearrange("p (a b) -> p a b", a=3)
            if rt % 2 == 0:
                nc.vector.tensor_copy(dstap, srcap)
            else:
                nc.scalar.activation(
                    dstap, srcap, mybir.ActivationFunctionType.Copy
                )

    x_src = x_d.rearrange("(rt p) c -> rt p c", p=P)
    wa_src = wa_d.rearrange("(rt p) c -> rt p c", p=P)
    wp_src = wp_d.rearrange("(rt p) c -> rt p c", p=P)

    # phase 0a: x tiles, then the V-column tiles of W_attn (rt 12-17)
    for rt in range(TT):
        transpose_tile(x_src[rt], xT, rt)
    for rt in range(12, 18):
        transpose_tile(wa_src[rt], WT, rt)

    # phase 0b/1a: V projection interleaved with Q/K weight transposes
    wqk_order = [0, 6, 1, 7, 2, 8, 3, 9, 4, 10, 5, 11]
    wi = 0
    for tt in range(TT):
        pss = psP.tile([P, 1024], F32, tag="pp", name="vps")
        for jb in range(2):
            for ct in range(CT):
                nc.tensor.matmul(
                    pss[:, jb * 512 : jb * 512 + 384],
                    xT[:, ct, tt * P : (tt + 1) * P],
                    WT[:, ct, 2 * C + jb * 384 : 2 * C + (jb + 1) * 384],
                    start=(ct == 0),
                    stop=(ct == CT - 1),
                )
        for jb in range(2):
            sl = slice(jb * 384, (jb + 1) * 384)
            nc.vector.tensor_add(
                V[:, tt, 6 * jb : 6 * jb + 6, 0:64],
                pss[:, jb * 512 : jb * 512 + 384].rearrange("p (h d) -> p h d", d=64),
                Bv[:, sl].rearrange("p (h d) -> p h d", d=64),
            )
        for _ in range(2):
            if wi < 12:
                transpose_tile(wa_src[wqk_order[wi]], WT, wqk_order[wi])
                wi += 1
    while wi < 12:
        transpose_tile(wa_src[wqk_order[wi]], WT, wqk_order[wi])
        wi += 1

    # ---- Q/K projection for head pair j: psum tiles + MM thunks + finish
    def qkproj_alloc(j):
        pQ = psP.tile([P, 1024], F32, tag="pp", name=f"pQ{j}")
        pK = psP.tile([P, 1024], F32, tag="pp", name=f"pK{j}")
        return (pQ, pK)

    def qkproj_mms(j, tiles):
        pQ, pK = tiles
        thunks = []
        for jt, pt in ((j, pQ), (6 + j, pK)):
            for tb in range(2):
                for ct in range(CT):
                    def mm(jt=jt, pt=pt, tb=tb, ct=ct):
                        nc.tensor.matmul(
                            pt[:, tb * 512 : (tb + 1) * 512],
                            WT[:, ct, jt * P : (jt + 1) * P],
                            xT[:, ct, tb * 512 : (tb + 1) * 512],
                            start=(ct == 0),
                            stop=(ct == CT - 1),
                        )
                    thunks.append(mm)
        return thunks

    def qkproj_finish(j, tiles):
        pQ, pK = tiles
        nc.vector.tensor_scalar_add(qT[:, j, :], pQ[:], bias_qk[:, j : j + 1])
        nc.vector.tensor_scalar_add(KTp[:, j, :], pK[:], bias_qk[:, 6 + j : 7 + j])

    # ---- phase 2 helpers
    def emit_qk_pair(j, sched, holder=None):
        """S^T and P^T for heads 2j (PE rows 0-63) and 2j+1 (rows 64-127),
        row-tiled concurrent. `sched` is a list of (after_kt, thunk):
        thunk is emitted once k-tile `after_kt` is fully emitted."""
        PTa = ptp.tile([P, KT, T], BF16, tag="PT", name=f"PTa{j}")
        PTb = ptp.tile([P, KT, T], BF16, tag="PT", name=f"PTb{j}")
        if holder is not None:
            holder["PTa"], holder["PTb"] = PTa, PTb
        si = 0
        for kt in range(KT):
            qs = kt * P
            for ci, (q0, w) in enumerate(qk_chunks(kt)):
                spsa = psS.tile([P, 512], F32, tag="sps", name="spsa")
                spsb = psS.tile([P, 512], F32, tag="sps", name="spsb")
                nc.tensor.matmul(
                    spsa[:, :w],
                    KTp[0:64, j, kt * P : (kt + 1) * P],
                    qT[0:64, j, q0 : q0 + w],
                    start=True, stop=True,
                    tile_position=(0, 0),
                )
                nc.tensor.matmul(
                    spsb[:, :w],
                    KTp[64:128, j, kt * P : (kt + 1) * P],
                    qT[64:128, j, q0 : q0 + w],
                    start=True, stop=True,
                    tile_position=(64, 0),
                )
                nc.scalar.activation(
                    PTa[:, kt, q0 : q0 + w], spsa[:, :w],
                    mybir.ActivationFunctionType.Exp, scale=0.125,
                )
                nc.scalar.activation(
                    PTb[:, kt, q0 : q0 + w], spsb[:, :w],
                    mybir.ActivationFunctionType.Exp, scale=0.125,
                )
                if ci == 0:
                    for PT in (PTa, PTb):
                        nc.gpsimd.affine_select(
                            out=PT[:, kt, qs : qs + P],
                            in_=PT[:, kt, qs : qs + P],
                            compare_op=mybir.AluOpType.is_ge,
                            fill=0.0,
                            base=0,
                            # keep where (-k + q') >= 0, else fill 0
                            pattern=[[1, P]],
                            channel_multiplier=-1,
                        )
                # weave scheduled thunks whose dependency k-tile is done
                while si < len(sched) and sched[si][0] < kt:
                    sched[si][1]()
                    si += 1
            while si < len(sched) and sched[si][0] <= kt:
                sched[si][1]()
                si += 1
        while si < len(sched):
            sched[si][1]()
            si += 1
        return PTa, PTb

    def emit_pv_head(h, PT, yD, blocks=(0, 1)):
        for b in blocks:
            for kt in range(KT):
                if b == 0 and kt >= 4:
                    continue
                off = max(0, kt * P - b * 512)
                w = 512 - off
                first = kt == 0
                last = kt == (3 if b == 0 else KT - 1)
                nc.tensor.matmul(
                    yD[0:65, b * 512 + off : b * 512 + off + w],
                    V[:, kt, h, :],
                    PT[:, kt, b * 512 + off : b * 512 + off + w],
                    start=first,
                    stop=last,
                )

    def emit_norm_head(h, yD, b0=0, b1=2):
        """yT_h[:, b0*512:b1*512] = yD[0:64] * (1/D), D = yD[64]; PE-free."""
        n = (b1 - b0) * 512
        sl = slice(b0 * 512, b1 * 512)
        Dsb = work.tile([65, T], F32, tag="Dsb")
        nc.vector.tensor_copy(Dsb[64:65, sl], yD[64:65, sl])
        Dd = dram.tile([T], F32, tag="Dd")
        nc.sync.dma_start(
            Dd[sl].rearrange("(a t) -> a t", a=1), Dsb[64:65, sl]
        )
        Dfull = work.tile([64, T], F32, tag="Dfull")
        nc.sync.dma_start(
            Dfull[0:64, sl],
            Dd[sl].rearrange("(a t) -> a t", a=1).to_broadcast([64, n]),
        )
        Rh = work.tile([64, T], F32, tag="Rh")
        nc.vector.reciprocal_approx_fast(Rh[0:64, sl], Dfull[0:64, sl])
        if h % 2 == 0:
            nc.vector.tensor_mul(yT[0:64, h // 2, sl], yD[0:64, sl], Rh[0:64, sl])
        else:
            yTt = work.tile([64, T], BF16, tag="yTt")
            nc.vector.tensor_mul(yTt[0:64, sl], yD[0:64, sl], Rh[0:64, sl])
            nc.sync.dma_start(yT[64:128, h // 2, sl], yTt[0:64, sl])

    def emit_outproj(tt):
        pss = psP.tile([P, 1024], F32, tag="pp", name="ops")
        for jb in range(2):
            for ct in range(CT):
                nc.tensor.matmul(
                    pss[:, jb * 512 : jb * 512 + 384],
                    yT[:, ct, tt * P : (tt + 1) * P],
                    WpT[:, ct, jb * 384 : (jb + 1) * 384],
                    start=(ct == 0),
                    stop=(ct == CT - 1),
                )
        osb = work.tile([P, C], F32, tag="osb")
        for jb in range(2):
            sl = slice(jb * 384, (jb + 1) * 384)
            nc.vector.tensor_add(osb[:, sl], pss[:, jb * 512 : jb * 512 + 384], Bp[:, sl])
        nc.sync.dma_start(y_d.rearrange("(tt p) c -> tt p c", p=P)[tt], osb[:])

    # ---- pair loop: Q/K proj of pair p+1 woven into pair p's QK stretch
    tiles0 = qkproj_alloc(0)
    for mm in qkproj_mms(0, tiles0):
        mm()
    qkproj_finish(0, tiles0)
    for p in range(6):
        if p + 1 < 6:
            nxt = qkproj_alloc(p + 1)
            mms = qkproj_mms(p + 1, nxt)
            # spread 24 proj MMs evenly across the 12 chunk points
            sched = [((i * KT) // len(mms), mm) for i, mm in enumerate(mms)]
            PTa, PTb = emit_qk_pair(p, sched)
            qkproj_finish(p + 1, nxt)
            yD0 = psP.tile([P, 1024], F32, tag="pp", name=f"yD{2*p}")
            emit_pv_head(2 * p, PTa, yD0)
            emit_norm_head(2 * p, yD0)
            yD1 = psP.tile([P, 1024], F32, tag="pp", name=f"yD{2*p+1}")
            emit_pv_head(2 * p + 1, PTb, yD1)
            emit_norm_head(2 * p + 1, yD1)
        else:
            # last pair: W_proj transposes fill early k-tiles; the b=0
            # halves of PV + normalization weave in after k-tile 3 so
            # their chains drain during the rest of the stretch.
            yD0 = psP.tile([P, 1024], F32, tag="pp", name="yD10")
            yD1 = psP.tile([P, 1024], F32, tag="pp", name="yD11")
            holder = {}
            sched = [
                (0, lambda: transpose_tile(wp_src[0], WpT, 0)),
                (0, lambda: transpose_tile(wp_src[1], WpT, 1)),
                (1, lambda: transpose_tile(wp_src[2], WpT, 2)),
                (1, lambda: transpose_tile(wp_src[3], WpT, 3)),
                (2, lambda: transpose_tile(wp_src[4], WpT, 4)),
                (2, lambda: transpose_tile(wp_src[5], WpT, 5)),
                (3, lambda: emit_pv_head(10, holder["PTa"], yD0, blocks=(0,))),
                (3, lambda: emit_norm_head(10, yD0, 0, 1)),
                (4, lambda: emit_pv_head(11, holder["PTb"], yD1, blocks=(0,))),
                (4, lambda: emit_norm_head(11, yD1, 0, 1)),
            ]
            PTa, PTb = emit_qk_pair(5, sched, holder=holder)
            emit_pv_head(10, PTa, yD0, blocks=(1,))
            emit_pv_head(11, PTb, yD1, blocks=(1,))
            emit_norm_head(10, yD0, 1, 2)
            emit_norm_head(11, yD1, 1, 2)
            for tt in range(TT):
                emit_outproj(tt)


def build_program(loop=1):
    nc = bacc.Bacc("TRN2", target_bir_lowering=False, debug=False)
    x_d = nc.dram_tensor("x", [T, C], F32, kind="ExternalInput").ap()
    wa_d = nc.dram_tensor("W_attn", [3 * C, C], F32, kind="ExternalInput").ap()
    ba_d = nc.dram_tensor("b_attn", [3 * C], F32, kind="ExternalInput").ap()
    wp_d = nc.dram_tensor("W_proj", [C, C], F32, kind="ExternalInput").ap()
    bp_d = nc.dram_tensor("b_proj", [C], F32, kind="ExternalInput").ap()
    y_d = nc.dram_tensor("y", [T, C], F32, kind="ExternalOutput").ap()

    with tile.TileContext(nc) as tc, ExitStack() as ctx:
        const = ctx.enter_context(tc.tile_pool(name="const", bufs=1))
        persist = ctx.enter_context(tc.tile_pool(name="persist", bufs=1))
        nat = ctx.enter_context(tc.tile_pool(name="nat", bufs=6))
        work = ctx.enter_context(tc.tile_pool(name="work", bufs=2))
        ptp = ctx.enter_context(tc.tile_pool(name="ptp", bufs=3))
        dram = ctx.enter_context(tc.tile_pool(name="dram", bufs=2, space="DRAM"))
        psS = ctx.enter_context(tc.tile_pool(name="psS", bufs=4, space="PSUM"))
        psP = ctx.enter_context(tc.tile_pool(name="psP", bufs=2, space="PSUM"))
        pools = (const, persist, nat, work, ptp, dram, psS, psP)

        cst = emit_consts(nc, tc, const, ba_d, bp_d)
        V0 = persist.tile([P, TT, 12, 65], BF16, tag="Vaug")
        nc.gpsimd.memset(V0[:, :, :, 64:65], 1.0)
        loop_cm = tc.For_i(0, loop, 1) if loop > 1 else contextlib.nullcontext()
        with loop_cm:
            emit_body(nc, tc, pools, cst, x_d, wa_d, wp_d, y_d)

    nc.compile()
    return nc


_CACHED_NC = None


def kernel(x, W_attn, b_attn, W_proj, b_proj):
    from concourse.bass_utils import run_bass_kernel_spmd

    global _CACHED_NC
    if _CACHED_NC is None:
        _CACHED_NC = build_program(loop=1)
    nc = _CACHED_NC

    B = x.shape[0]
    assert B == N_CORES
    in_maps = [
        {
            "x": np.ascontiguousarray(x[b], dtype=np.float32),
            "W_attn": np.asarray(W_attn, dtype=np.float32),
            "b_attn": np.asarray(b_attn, dtype=np.float32),
            "W_proj": np.asarray(W_proj, dtype=np.float32),
            "b_proj": np.asarray(b_proj, dtype=np.float32),
        }
        for b in range(B)
    ]
    res = run_bass_kernel_spmd(nc, in_maps, list(range(N_CORES)))
    return np.stack([res.results[b]["y"] for b in range(B)], axis=0)


# revision 26
# speedup vs baseline: 1.7588x; 1.1063x over previous
"""Causal self-attention kernel for Trainium2, 8-core data parallel.

Per-core program: one batch element b of x [8, 1024, 768].

Software-pipelined structure: for each head pair p the "stretch" emits
pair p's row-tiled QK matmuls + exps (ACT engine is the pacer), weaving
between them as PE filler: the PV matmuls of pair p-1 and the Q/K
projection matmuls of pair p+1 (K psum tile first half, Q tile second
half, so only one projection tile is live at a time).  V projection
fills stretch 0; W_proj transposes fill stretch 5.  Normalization runs
entirely off the PE: ones-column denominator row -> DRAM -> broadcast
read -> DVE reciprocal_approx_fast -> DVE multiply.

PSUM (8 banks): psS ring 2x[128,512] (transients: QK S-tiles, woven
transposes / V-proj) + psP ring 3x[128,1024] (pK/pQ projection tiles,
yD PV accumulators, phase-0 transposes, out-proj).  Careful allocation
order keeps every ring-slot wait's producer earlier in program order.
All matmuls bf16 / fp32 PSUM; softmax fp32; causal P^T tiles stored
flattened (only q >= kt*128 kept).
"""
import sys
import contextlib
from contextlib import ExitStack

sys.path.insert(0, "/opt/trn_rl_repo")

import numpy as np

import concourse.bass as bass
import concourse.bacc as bacc
import concourse.mybir as mybir
import concourse.tile as tile

F32 = mybir.dt.float32
BF16 = mybir.dt.bfloat16
P = 128
T = 1024
C = 768
H = 12
HS = 64
CT = C // P
TT = T // P
KT = T // P
N_CORES = 8

# flat offsets for causal P^T storage: k-tile kt keeps q in [kt*128, 1024)
PT_OFF = [0]
for _kt in range(1, KT + 1):
    PT_OFF.append(PT_OFF[-1] + (T - 128 * (_kt - 1)))
PT_W = PT_OFF[-1]  # 4608


def qk_chunks(kt):
    qs = kt * P
    if kt < 4:
        return [(qs, 512 - qs), (512, 512)]
    return [(qs, T - qs)]


def emit_consts(nc, tc, const, ba_d, bp_d):
    from concourse.masks import make_identity
    ident = const.tile([P, P], F32, tag="ident")
    make_identity(nc, ident)
    bias_qk = const.tile([P, 12], F32, tag="bqk")
    nc.sync.dma_start(bias_qk[:], ba_d[0 : 2 * C].rearrange("(o p) -> p o", p=P))
    Bv = const.tile([P, C], F32, tag="Bv")
    nc.sync.dma_start(
        Bv[:],
        ba_d[2 * C : 3 * C].rearrange("(a j) -> a j", a=1).to_broadcast([P, C]),
    )
    Bp = const.tile([P, C], F32, tag="Bp")
    nc.sync.dma_start(
        Bp[:], bp_d.rearrange("(a j) -> a j", a=1).to_broadcast([P, C])
    )
    return dict(ident=ident, bias_qk=bias_qk, Bv=Bv, Bp=Bp)


def emit_body(nc, tc, pools, cst, x_d, wa_d, wp_d, y_d):
    const, persist, nat, work, ptp, dram, psS, psP = pools
    bias_qk, Bv, Bp = cst["bias_qk"], cst["Bv"], cst["Bp"]
    ident = cst["ident"]

    xT = persist.tile([P, CT, T], BF16, tag="xT")
    WT = persist.tile([P, CT, 3 * C], BF16, tag="WT")
    WpT = persist.tile([P, CT, C], BF16, tag="WpT")
    qT = persist.tile([P, 6, T], BF16, tag="qT")
    KTp = persist.tile([P, 6, T], BF16, tag="KTp")
    V = persist.tile([P, TT, 12, 65], BF16, tag="Vaug")
    yT = persist.tile([P, CT, T], BF16, tag="yT")

    # ---- PE transposes, split into DMA-load and compute parts
    def transpose_load(dram_src_rt):
        natt = nat.tile([P, C], F32, tag="nat")
        nc.sync.dma_start(natt[:], dram_src_rt)
        return natt

    def transpose_compute(natt, dst, rt, in_stretch):
        for g in range(2):
            if in_stretch:
                ps = psS.tile([P, 512], F32, tag="sps", name="tps")
                base = 0
            else:
                if g == 0:
                    psbig = psP.tile([P, 1024], F32, tag="pp", name="tps")
                ps = psbig
                base = g * 512
            for i in range(3):
                ct = g * 3 + i
                nc.tensor.transpose(
                    ps[:, base + i * P : base + (i + 1) * P],
                    natt[:, ct * P : (ct + 1) * P],
                    ident[:],
                )
            dstap = dst[:, g * 3 : (g + 1) * 3, rt * P : (rt + 1) * P]
            srcap = ps[:, base : base + 384].rearrange("p (a b) -> p a b", a=3)
            if (rt + g) % 2 == 0:
                nc.vector.tensor_copy(dstap, srcap)
            else:
                nc.scalar.activation(
                    dstap, srcap, mybir.ActivationFunctionType.Copy
                )

    x_src = x_d.rearrange("(rt p) c -> rt p c", p=P)
    wa_src = wa_d.rearrange("(rt p) c -> rt p c", p=P)
    wp_src = wp_d.rearrange("(rt p) c -> rt p c", p=P)

    # ---- Q/K projection pieces (one PSUM tile at a time: K then Q)
    def proj_alloc(name):
        return psP.tile([P, 1024], F32, tag="pp", name=name)

    def proj_mms(jt, pt):
        thunks = []
        for tb in range(2):
            for ct in range(CT):
                def mm(jt=jt, pt=pt, tb=tb, ct=ct):
                    nc.tensor.matmul(
                        pt[:, tb * 512 : (tb + 1) * 512],
                        WT[:, ct, jt * P : (jt + 1) * P],
                        xT[:, ct, tb * 512 : (tb + 1) * 512],
                        start=(ct == 0),
                        stop=(ct == CT - 1),
                    )
                thunks.append(mm)
        return thunks

    def proj_copy(j, jt, pt):
        dst = qT if jt < 6 else KTp
        nc.vector.tensor_scalar_add(dst[:, j, :], pt[:], bias_qk[:, jt : jt + 1])

    # ---- V projection for one tt (psS transient tiles, one jb at a time)
    def vproj_thunk(tt):
        def go():
            for jb in range(2):
                ps = psS.tile([P, 512], F32, tag="sps", name="vps")
                for ct in range(CT):
                    nc.tensor.matmul(
                        ps[:, :384],
                        xT[:, ct, tt * P : (tt + 1) * P],
                        WT[:, ct, 2 * C + jb * 384 : 2 * C + (jb + 1) * 384],
                        start=(ct == 0),
                        stop=(ct == CT - 1),
                    )
                sl = slice(jb * 384, (jb + 1) * 384)
                nc.vector.tensor_add(
                    V[:, tt, 6 * jb : 6 * jb + 6, 0:64],
                    ps[:, :384].rearrange("p (h d) -> p h d", d=64),
                    Bv[:, sl].rearrange("p (h d) -> p h d", d=64),
                )
        return go

    # ---- PV matmul thunks (kt-granular, accumulate into yD)
    def pv_thunks(h, PT, yD, blocks=(0, 1)):
        thunks = []
        for b in blocks:
            for kt in range(KT):
                if b == 0 and kt >= 4:
                    continue
                off = max(0, kt * P - b * 512)
                w = 512 - off
                first = kt == 0
                last = kt == (3 if b == 0 else KT - 1)
                q0 = b * 512 + off
                fo = PT_OFF[kt] + (q0 - kt * P)
                def mm(h=h, PT=PT, yD=yD, q0=q0, w=w, fo=fo,
                       first=first, last=last, kt=kt):
                    nc.tensor.matmul(
                        yD[0:65, q0 : q0 + w],
                        V[:, kt, h, :],
                        PT[:, fo : fo + w],
                        start=first,
                        stop=last,
                    )
                thunks.append(mm)
        return thunks

    def emit_norm_head(h, yD, b0=0, b1=2):
        n = (b1 - b0) * 512
        sl = slice(b0 * 512, b1 * 512)
        Dsb = work.tile([65, T], F32, tag="Dsb")
        nc.vector.tensor_copy(Dsb[64:65, sl], yD[64:65, sl])
        Dd = dram.tile([T], F32, tag="Dd")
        nc.sync.dma_start(Dd[sl].rearrange("(a t) -> a t", a=1), Dsb[64:65, sl])
        Dfull = work.tile([64, T], F32, tag="Dfull")
        nc.sync.dma_start(
            Dfull[0:64, sl],
            Dd[sl].rearrange("(a t) -> a t", a=1).to_broadcast([64, n]),
        )
        Rh = work.tile([64, T], F32, tag="Rh")
        nc.vector.reciprocal_approx_fast(Rh[0:64, sl], Dfull[0:64, sl])
        if h % 2 == 0:
            nc.vector.tensor_mul(yT[0:64, h // 2, sl], yD[0:64, sl], Rh[0:64, sl])
        else:
            yTt = work.tile([64, T], BF16, tag="yTt")
            nc.vector.tensor_mul(yTt[0:64, sl], yD[0:64, sl], Rh[0:64, sl])
            nc.sync.dma_start(yT[64:128, h // 2, sl], yTt[0:64, sl])

    def emit_outproj(tt):
        pss = psP.tile([P, 1024], F32, tag="pp", name="ops")
        for jb in range(2):
            for ct in range(CT):
                nc.tensor.matmul(
                    pss[:, jb * 512 : jb * 512 + 384],
                    yT[:, ct, tt * P : (tt + 1) * P],
                    WpT[:, ct, jb * 384 : (jb + 1) * 384],
                    start=(ct == 0),
                    stop=(ct == CT - 1),
                )
        osb = work.tile([P, C], F32, tag="osb")
        for jb in range(2):
            sl = slice(jb * 384, (jb + 1) * 384)
            nc.vector.tensor_add(osb[:, sl], pss[:, jb * 512 : jb * 512 + 384], Bp[:, sl])
        nc.sync.dma_start(y_d.rearrange("(tt p) c -> tt p c", p=P)[tt], osb[:])

    # ---- the QK stretch for pair j with first/second-half filler queues
    def emit_stretch(j, first_half, second_half):
        PTa = ptp.tile([P, PT_W], BF16, tag="PT", name=f"PTa{j}")
        PTb = ptp.tile([P, PT_W], BF16, tag="PT", name=f"PTb{j}")
        points = sum(len(qk_chunks(kt)) for kt in range(KT))  # 12
        half_pt = points // 2
        pi = 0
        f1 = f2 = 0
        for kt in range(KT):
            qs = kt * P
            for ci, (q0, w) in enumerate(qk_chunks(kt)):
                spsa = psS.tile([P, 512], F32, tag="sps", name="spsa")
                spsb = psS.tile([P, 512], F32, tag="sps", name="spsb")
                nc.tensor.matmul(
                    spsa[:, :w],
                    KTp[0:64, j, kt * P : (kt + 1) * P],
                    qT[0:64, j, q0 : q0 + w],
                    start=True, stop=True,
                    tile_position=(0, 0),
                )
                nc.tensor.matmul(
                    spsb[:, :w],
                    KTp[64:128, j, kt * P : (kt + 1) * P],
                    qT[64:128, j, q0 : q0 + w],
                    start=True, stop=True,
                    tile_position=(64, 0),
                )
                fo = PT_OFF[kt] + (q0 - qs)
                nc.scalar.activation(
                    PTa[:, fo : fo + w], spsa[:, :w],
                    mybir.ActivationFunctionType.Exp, scale=0.125,
                )
                nc.scalar.activation(
                    PTb[:, fo : fo + w], spsb[:, :w],
                    mybir.ActivationFunctionType.Exp, scale=0.125,
                )
                if ci == 0:
                    dg = PT_OFF[kt]
                    for PT in (PTa, PTb):
                        nc.gpsimd.affine_select(
                            out=PT[:, dg : dg + P],
                            in_=PT[:, dg : dg + P],
                            compare_op=mybir.AluOpType.is_ge,
                            fill=0.0,
                            base=0,
                            pattern=[[1, P]],
                            channel_multiplier=-1,
                        )
                pi += 1
                if pi <= half_pt:
                    want = (len(first_half) * pi) // half_pt
                    while f1 < want:
                        first_half[f1]()
                        f1 += 1
                else:
                    want = (len(second_half) * (pi - half_pt)) // (points - half_pt)
                    while f2 < want:
                        second_half[f2]()
                        f2 += 1
        while f1 < len(first_half):
            first_half[f1]()
            f1 += 1
        while f2 < len(second_half):
            second_half[f2]()
            f2 += 1
        return PTa, PTb

    # ================= prologue: transposes + pair-0 projection =================
    # order: x (8), W rt 0 and 6 (pair-0 deps), V-cols 12-17, rest Q/K cols
    w_order = [0, 6, 12, 13, 14, 15, 16, 17, 1, 7, 2, 8, 3, 9, 4, 10, 5, 11]
    seq = [(x_src[rt], xT, rt) for rt in range(TT)]
    seq += [(wa_src[rt], WT, rt) for rt in w_order]
    PREF = 4
    emitted = 0
    pending = []

    def pump(n_compute, in_stretch=False):
        nonlocal emitted, pending
        while len(pending) < PREF and emitted < len(seq):
            s = seq[emitted]
            pending.append((transpose_load(s[0]), s[1], s[2]))
            emitted += 1
        for _ in range(n_compute):
            if not pending:
                return
            natt, dst, rt = pending.pop(0)
            transpose_compute(natt, dst, rt, in_stretch)
            while len(pending) < PREF and emitted < len(seq):
                s = seq[emitted]
                pending.append((transpose_load(s[0]), s[1], s[2]))
                emitted += 1

    pump(10)  # x tiles + W rt 0, 6 (psP tiles, transient)
    # pair-0 projection woven with the remaining 16 transposes; the live
    # pK0/pQ0 tiles sit in psP, so the woven transposes use psS tiles.
    pK0 = proj_alloc("pK0")
    mmsK0 = proj_mms(6, pK0)
    state = {"q": None, "pQ": None, "i": 0}
    for step in range(16):
        pump(1, in_stretch=True)
        for _ in range(2):
            if state["q"] is None:
                if state["i"] < len(mmsK0):
                    mmsK0[state["i"]]()
                    state["i"] += 1
                else:
                    proj_copy(0, 6, pK0)
                    state["pQ"] = proj_alloc("pQ0")
                    state["q"] = proj_mms(0, state["pQ"])
                    state["i"] = 0
            else:
                if state["i"] < len(state["q"]):
                    state["q"][state["i"]]()
                    state["i"] += 1
    if state["q"] is None:
        proj_copy(0, 6, pK0)
        state["pQ"] = proj_alloc("pQ0")
        state["q"] = proj_mms(0, state["pQ"])
        state["i"] = 0
    while state["i"] < len(state["q"]):
        state["q"][state["i"]]()
        state["i"] += 1
    proj_copy(0, 0, state["pQ"])

    # ================= pair loop =================
    PTs = {}
    for p in range(6):
        if p == 0:
            # stretch 0: V projection (psS transients) + pair-1 projections
            pK = proj_alloc("pK1")
            projK = proj_mms(7, pK)
            first = []
            for i in range(4):
                first.append(vproj_thunk(i))
                first.append(projK[2 * i])
                first.append(projK[2 * i + 1])
            first += projK[8:]
            holder = {}
            def mid0(pK=pK):
                proj_copy(1, 7, pK)
                holder["pQ"] = proj_alloc("pQ1")
                holder["q"] = proj_mms(1, holder["pQ"])
            second = [mid0]
            for i in range(4, TT):
                second.append(vproj_thunk(i))
                second.append(lambda i=i: holder["q"][2 * (i - 4)]())
                second.append(lambda i=i: holder["q"][2 * (i - 4) + 1]())
            second += [lambda k=k: holder["q"][k]() for k in range(8, 12)]
            second.append(lambda: proj_copy(1, 1, holder["pQ"]))
            PTa, PTb = emit_stretch(0, first, second)
            PTs[0], PTs[1] = PTa, PTb
            continue
        ha, hb = 2 * p - 2, 2 * p - 1
        PTa_, PTb_ = PTs.pop(ha), PTs.pop(hb)
        if p < 5:
            jn = p + 1
            pK = proj_alloc(f"pK{jn}")
            yDa = psP.tile([P, 1024], F32, tag="pp", name=f"yD{ha}")
            projK = proj_mms(6 + jn, pK)
            pva = pv_thunks(ha, PTa_, yDa)
            first = []
            for i in range(12):
                first.append(projK[i])
                first.append(pva[i])
            holder = {}
            def mid(jn=jn, pK=pK, ha=ha, yDa=yDa):
                proj_copy(jn, 6 + jn, pK)
                holder["pQ"] = proj_alloc(f"pQ{jn}")
                holder["yDb"] = psP.tile([P, 1024], F32, tag="pp",
                                         name=f"yD{ha+1}")
                holder["q"] = proj_mms(jn, holder["pQ"])
                holder["pvb"] = pv_thunks(ha + 1, PTb_, holder["yDb"])
                emit_norm_head(ha, yDa)
            second = [mid]
            for i in range(12):
                second.append(lambda i=i: holder["q"][i]())
                second.append(lambda i=i: holder["pvb"][i]())
            def tail(jn=jn, hb=hb):
                proj_copy(jn, jn, holder["pQ"])
                emit_norm_head(hb, holder["yDb"])
            second.append(tail)
            PTa, PTb = emit_stretch(p, first, second)
            PTs[2 * p], PTs[2 * p + 1] = PTa, PTb
        else:
            # stretch 5: W_proj transposes (psS transients) + PV(pair 4)
            yDa = psP.tile([P, 1024], F32, tag="pp", name="yD8")
            wp_loads = [transpose_load(wp_src[rt]) for rt in range(3)]
            pva = pv_thunks(8, PTa_, yDa)
            first = []
            for i in range(12):
                first.append(pva[i])
                if i % 4 == 0:
                    first.append(lambda rt=i // 4, n=wp_loads[i // 4]:
                                 transpose_compute(n, WpT, rt, True))
            holder = {}
            def mid5(yDa=yDa):
                emit_norm_head(8, yDa)
                holder["yDb"] = psP.tile([P, 1024], F32, tag="pp", name="yD9")
                holder["loads2"] = [transpose_load(wp_src[rt]) for rt in (3, 4, 5)]
                holder["pvb"] = pv_thunks(9, PTb_, holder["yDb"])
            second = [mid5]
            for i in range(12):
                second.append(lambda i=i: holder["pvb"][i]())
                if i % 4 == 0:
                    second.append(lambda i=i: transpose_compute(
                        holder["loads2"][i // 4], WpT, 3 + i // 4, True))
            second.append(lambda: emit_norm_head(9, holder["yDb"]))
            PTa, PTb = emit_stretch(5, first, second)
            # epilogue: pair-5 PV with per-half norms, then out-proj
            yD10 = psP.tile([P, 1024], F32, tag="pp", name="yD10")
            yD11 = psP.tile([P, 1024], F32, tag="pp", name="yD11")
            for t in pv_thunks(10, PTa, yD10, blocks=(0,)):
                t()
            emit_norm_head(10, yD10, 0, 1)
            for t in pv_thunks(11, PTb, yD11, blocks=(0,)):
                t()
            emit_norm_head(11, yD11, 0, 1)
            for t in pv_thunks(10, PTa, yD10, blocks=(1,)):
                t()
            emit_norm_head(10, yD10, 1, 2)
            for t in pv_thunks(11, PTb, yD11, blocks=(1,)):
                t()
            emit_norm_head(11, yD11, 1, 2)
            for tt in range(TT):
                emit_outproj(tt)


def build_program(loop=1):
    nc = bacc.Bacc("TRN2", target_bir_lowering=False, debug=False)
    x_d = nc.dram_tensor("x", [T, C], F32, kind="ExternalInput").ap()
    wa_d = nc.dram_tensor("W_attn", [3 * C, C], F32, kind="ExternalInput").ap()
    ba_d = nc.dram_tensor("b_attn", [3 * C], F32, kind="ExternalInput").ap()
    wp_d = nc.dram_tensor("W_proj", [C, C], F32, kind="ExternalInput").ap()
    bp_d = nc.dram_tensor("b_proj", [C], F32, kind="ExternalInput").ap()
    y_d = nc.dram_tensor("y", [T, C], F32, kind="ExternalOutput").ap()

    with tile.TileContext(nc) as tc, ExitStack() as ctx:
        const = ctx.enter_context(tc.tile_pool(name="const", bufs=1))
        persist = ctx.enter_context(tc.tile_pool(name="persist", bufs=1))
        nat = ctx.enter_context(tc.tile_pool(name="nat", bufs=6))
        work = ctx.enter_context(tc.tile_pool(name="work", bufs=2))
        ptp = ctx.enter_context(tc.tile_pool(name="ptp", bufs=4))
        dram = ctx.enter_context(tc.tile_pool(name="dram", bufs=2, space="DRAM"))
        psS = ctx.enter_context(tc.tile_pool(name="psS", bufs=2, space="PSUM"))
        psP = ctx.enter_context(tc.tile_pool(name="psP", bufs=3, space="PSUM"))
        pools = (const, persist, nat, work, ptp, dram, psS, psP)

        cst = emit_consts(nc, tc, const, ba_d, bp_d)
        V0 = persist.tile([P, TT, 12, 65], BF16, tag="Vaug")
        nc.gpsimd.memset(V0[:, :, :, 64:65], 1.0)
        loop_cm = tc.For_i(0, loop, 1) if loop > 1 else contextlib.nullcontext()
        with loop_cm:
            emit_body(nc, tc, pools, cst, x_d, wa_d, wp_d, y_d)

    nc.compile()
    return nc


_CACHED_NC = None


def kernel(x, W_attn, b_attn, W_proj, b_proj):
    from concourse.bass_utils import run_bass_kernel_spmd

    global _CACHED_NC
    if _CACHED_NC is None:
        _CACHED_NC = build_program(loop=1)
    nc = _CACHED_NC

    B = x.shape[0]
    assert B == N_CORES
    in_maps = [
        {
            "x": np.ascontiguousarray(x[b], dtype=np.float32),
            "W_attn": np.asarray(W_attn, dtype=np.float32),
            "b_attn": np.asarray(b_attn, dtype=np.float32),
            "W_proj": np.asarray(W_proj, dtype=np.float32),
            "b_proj": np.asarray(b_proj, dtype=np.float32),
        }
        for b in range(B)
    ]
    res = run_bass_kernel_spmd(nc, in_maps, list(range(N_CORES)))
    return np.stack([res.results[b]["y"] for b in range(B)], axis=0)
